# revision 1
# baseline (speedup 1.0000x reference)
"""Exponential smoothing (linear recurrence scan) on 8 trn2 NeuronCores.

Math (per batch b, head h, dim d):
    alpha = sigmoid(smoothing_weight[h])
    u[t]  = (1-alpha)*values[t] + factor*alpha*aux_values[t]
    y[t]  = alpha*y[t-1] + u[t],   y[-1] = v0
Sharding: data-parallel over batch b -> 8 cores, one batch each.

Device algorithm (per core, T=4096, HD=H*D=512), all fp32 data:
  - T is split into NG groups x CPG chunks of 128 rows.
  - Main: per head, Y_local = (c1*L)^T-matmul(v) + (c2*L)^T-matmul(a) where
    L[p,q] = alpha^(p-q) (p>=q) is the within-chunk scan matrix.  Chunks are
    batched along the matmul free dim (float32r -> 1 cycle/row at N>=256).
  - Level-2: chunk summaries s_c = Y_local[c][127] are scanned across chunks
    with small per-head matmuls using A = alpha^128 power matrices, yielding
    the carry P_c = S_{c-1} entering each chunk (S_{-1} = v0).
  - Fixup: rank-1 matmul decay_h (x) P_row accumulated into the output:
    y[128c+p] = Y_local[c][p] + alpha^(p+1) * P_c.
Cross-partition moves (chunk summaries -> partition-per-chunk, carries ->
single row) are done with small SBUF->SBUF DMAs.
"""

import sys

sys.path.insert(0, "/opt/trn_rl_repo")

import numpy as np

import concourse.bass as bass
import concourse.bacc as bacc
import concourse.mybir as mybir
from concourse.tile import TileContext
from concourse.bass_utils import run_bass_kernel_spmd

B, T, H, D = 8, 4096, 8, 64
HD = H * D                  # 512
P = 128                     # chunk length / partitions
NCHUNK = T // P             # 32
CPG = 8                     # chunks per group
NG = NCHUNK // CPG          # 4 groups
GT = CPG * P                # 1024 rows per group

F32 = mybir.dt.float32
F32R = mybir.dt.float32r


def build_consts(smoothing_weight, v0):
    """Host-side constant tensors (float64 math, cast to fp32)."""
    a = 1.0 / (1.0 + np.exp(-smoothing_weight.astype(np.float64).reshape(H)))
    c1 = 1.0 - a
    factor = c1 / np.maximum(c1, 1e-6)
    c2 = factor * a

    q = np.arange(P)
    e = q[None, :] - q[:, None]                     # [q, p] -> p - q
    pow_ = np.where(e >= 0, a[:, None, None] ** np.maximum(e, 0), 0.0)  # [h,q,p]
    w1 = (c1[:, None, None] * pow_).transpose(1, 0, 2).reshape(P, H * P)
    w2 = (c2[:, None, None] * pow_).transpose(1, 0, 2).reshape(P, H * P)

    decay = (a[:, None] ** (q[None, :] + 1)).reshape(1, H * P)  # [1, h*128]

    A = a ** P                                       # alpha^128 per head
    r = np.arange(CPG + 1)
    # augmented lhsT [9,9]: rows j=0..7 -> A^(r-1-j) for j<=r-1; row 8 -> A^r
    ee = (r[None, :] - 1) - np.arange(CPG)[:, None]  # [j, r]
    mexc = np.where(ee >= 0, A[:, None, None] ** np.maximum(ee, 0), 0.0)
    crow = (A[:, None] ** r[None, :])[:, None, :]
    mexc = np.concatenate([mexc, crow], axis=1)      # [h,9,9]
    mexc = mexc.transpose(1, 0, 2).reshape(CPG + 1, H * (CPG + 1))

    v0row = v0.astype(np.float64).reshape(1, HD)

    f = np.float32
    return {
        "w1": np.ascontiguousarray(w1, dtype=f),
        "w2": np.ascontiguousarray(w2, dtype=f),
        "decay": np.ascontiguousarray(decay, dtype=f),
        "mexc": np.ascontiguousarray(mexc, dtype=f),
        "v0r": np.ascontiguousarray(v0row, dtype=f),
    }


def build_nc():
    nc = bacc.Bacc()

    v_d = nc.declare_dram_parameter("v", [T, HD], F32R, isOutput=False)
    a_d = nc.declare_dram_parameter("a", [T, HD], F32R, isOutput=False)
    w1_d = nc.declare_dram_parameter("w1", [P, H * P], F32R, isOutput=False)
    w2_d = nc.declare_dram_parameter("w2", [P, H * P], F32R, isOutput=False)
    dec_d = nc.declare_dram_parameter("decay", [1, H * P], F32R, isOutput=False)
    mex_d = nc.declare_dram_parameter("mexc", [CPG + 1, H * (CPG + 1)], F32R,
                                      isOutput=False)
    v0_d = nc.declare_dram_parameter("v0r", [1, HD], F32R, isOutput=False)
    y_d = nc.declare_dram_parameter("y", [T, HD], F32, isOutput=True)

    with TileContext(nc) as tc:
        with (
            tc.tile_pool(name="wpool", bufs=1) as wpool,
            tc.tile_pool(name="vin", bufs=3) as vin,
            tc.tile_pool(name="ain", bufs=3) as ain,
            tc.tile_pool(name="yout", bufs=3) as yout,
            tc.tile_pool(name="small", bufs=2) as small,
            tc.tile_pool(name="psA", bufs=4, space="PSUM") as psA_pool,
            tc.tile_pool(name="psP", bufs=1, space="PSUM") as psP_pool,
            tc.tile_pool(name="psB", bufs=3, space="PSUM") as psB_pool,
        ):
            # constants -> SBUF once
            w1 = wpool.tile([P, H * P], F32R, tag="w1")
            w2 = wpool.tile([P, H * P], F32R, tag="w2")
            dec = wpool.tile([1, H * P], F32R, tag="dec")
            mex = wpool.tile([CPG + 1, H * (CPG + 1)], F32R, tag="mex")
            nc.scalar.dma_start(w1[:], w1_d[:])
            nc.scalar.dma_start(w2[:], w2_d[:])
            nc.scalar.dma_start(dec[:], dec_d[:])
            nc.scalar.dma_start(mex[:], mex_d[:])

            p_sb_prev = None

            for g in range(NG):
                r0 = g * GT
                # ---- stream group inputs:  [GT, HD] -> [P, CPG, HD]
                v_sb = vin.tile([P, CPG * HD], F32R, tag="v")
                a_sb = ain.tile([P, CPG * HD], F32R, tag="a")
                v3d = v_sb[:].rearrange("p (c m) -> p c m", c=CPG)
                a3d = a_sb[:].rearrange("p (c m) -> p c m", c=CPG)
                # 1MB half-loads: finer in/out interleave on the DMA ring,
                # and group 0's compute starts after half a group
                HG = GT // 2
                C4 = CPG // 2
                for half in range(2):
                    hr = r0 + half * HG
                    cs = slice(half * C4, (half + 1) * C4)
                    nc.sync.dma_start(
                        v3d[:, cs, :],
                        v_d[hr:hr + HG, :].rearrange("(c p) m -> p c m",
                                                     c=C4, p=P))
                    nc.sync.dma_start(
                        a3d[:, cs, :],
                        a_d[hr:hr + HG, :].rearrange("(c p) m -> p c m",
                                                     c=C4, p=P))

                y_sb = yout.tile([P, CPG * HD], F32, tag="y")
                v3 = v_sb[:].rearrange("p (c m) -> p c m", c=CPG)
                a3 = a_sb[:].rearrange("p (c m) -> p c m", c=CPG)
                y3 = y_sb[:].rearrange("p (c m) -> p c m", c=CPG)

                # ---- main within-chunk scan, per head
                for h in range(H):
                    psA = psA_pool.tile([P, CPG * D], F32, tag="psA")
                    rhs_v = v3[:, :, h * D:(h + 1) * D]   # [P, CPG, D]
                    rhs_a = a3[:, :, h * D:(h + 1) * D]
                    if g == 0:
                        C4 = CPG // 2
                        for half in range(2):
                            cs = slice(half * C4, (half + 1) * C4)
                            fs = slice(half * C4 * D, (half + 1) * C4 * D)
                            nc.tensor.matmul(psA[:, fs],
                                             w1[:, h * P:(h + 1) * P],
                                             rhs_v[:, cs, :],
                                             start=True, stop=False)
                            nc.tensor.matmul(psA[:, fs],
                                             w2[:, h * P:(h + 1) * P],
                                             rhs_a[:, cs, :],
                                             start=False, stop=True)
                    else:
                        nc.tensor.matmul(psA[:], w1[:, h * P:(h + 1) * P],
                                         rhs_v, start=True, stop=False)
                        nc.tensor.matmul(psA[:], w2[:, h * P:(h + 1) * P],
                                         rhs_a, start=False, stop=True)
                    # evacuate into interleaved (c, h, d) layout
                    nc.scalar.copy(y3[:, :, h * D:(h + 1) * D],
                                   psA[:].rearrange("p (c d) -> p c d", c=CPG))

                # ---- gather chunk summaries: row 127 -> [CPG, HD];
                #      row 8 = incoming carry state
                sT = small.tile([CPG + 1, HD], F32, tag="sT")
                nc.gpsimd.dma_start(sT[0:CPG, :], y_sb[P - 1:P, :])
                if g == 0:
                    nc.gpsimd.dma_start(sT[CPG:CPG + 1, :].bitcast(F32R),
                                        v0_d[:])
                else:
                    nc.gpsimd.dma_start(sT[CPG:CPG + 1, :],
                                        p_sb_prev[CPG:CPG + 1, :])

                # ---- level-2 scan: one augmented F32R matmul per head
                psP = psP_pool.tile([CPG + 1, HD], F32, tag="psP")
                for h in range(H):
                    hs = slice(h * (CPG + 1), (h + 1) * (CPG + 1))
                    nc.tensor.matmul(psP[:, h * D:(h + 1) * D], mex[:, hs],
                                     sT[:, h * D:(h + 1) * D].bitcast(F32R),
                                     start=True, stop=True)
                p_sb = small.tile([CPG + 1, HD], F32, tag="p_sb")
                nc.scalar.copy(p_sb[:], psP[:])

                # ---- scatter carries to a single row [1, (c, h, d)]
                prow = small.tile([1, CPG * HD], F32, tag="prow")
                nc.gpsimd.dma_start(prow[:], p_sb[0:CPG, :])

                # ---- fixup: rank-1 decay (x) carry, then add into y
                prow3 = prow[:].rearrange("o (c m) -> o c m", c=CPG)
                C4 = CPG // 2
                for h in range(H):
                    psB = psB_pool.tile([P, CPG * D], F32, tag="psB")
                    nc.tensor.matmul(psB[:], dec[0:1, h * P:(h + 1) * P],
                                     prow3[:, :, h * D:(h + 1) * D].bitcast(F32R),
                                     start=True, stop=True)
                    pb3 = psB[:].rearrange("p (c d) -> p c d", c=CPG)
                    for half in range(2):
                        cs = slice(half * C4, (half + 1) * C4)
                        yv = y3[:, cs, h * D:(h + 1) * D]
                        nc.vector.tensor_add(yv, yv, pb3[:, cs, :])
                # store per chunk-half: earlier, smoother out-stream
                for half in range(2):
                    cs = slice(half * C4, (half + 1) * C4)
                    hr = r0 + half * (GT // 2)
                    dst = y_d[hr:hr + GT // 2, :].rearrange(
                        "(c p) m -> p c m", c=C4, p=P)
                    nc.scalar.dma_start(dst, y3[:, cs, :])

                p_sb_prev = p_sb

    nc.finalize()
    return nc


_NC_CACHE = None


def _get_nc():
    global _NC_CACHE
    if _NC_CACHE is None:
        _NC_CACHE = build_nc()
    return _NC_CACHE


def kernel(values, aux_values, v0, smoothing_weight):
    consts = build_consts(smoothing_weight, v0)
    nc = _get_nc()
    in_maps = []
    for b in range(B):
        m = dict(consts)
        m["v"] = np.ascontiguousarray(values[b].reshape(T, HD), dtype=np.float32)
        m["a"] = np.ascontiguousarray(aux_values[b].reshape(T, HD), dtype=np.float32)
        in_maps.append(m)
    res = run_bass_kernel_spmd(nc, in_maps, list(range(B))).results
    out = np.stack([res[b]["y"].reshape(T, H, D) for b in range(B)])
    return out.astype(np.float32)



# revision 2
# speedup vs baseline: 1.1970x; 1.1970x over previous
"""Exponential smoothing (linear recurrence scan) on 8 trn2 NeuronCores.

Math (per batch b, head h, dim d):
    alpha = sigmoid(smoothing_weight[h])
    u[t]  = (1-alpha)*values[t] + factor*alpha*aux_values[t]
    y[t]  = alpha*y[t-1] + u[t],   y[-1] = v0
Sharding: data-parallel over batch b -> 8 cores, one batch each.

v2: bandwidth-optimized.  All HBM I/O is bf16 (12 MB/core instead of 24) and
the host pre-packs inputs into the device-native layout [P, chunk, HD]
(partition = time-within-chunk) so every DMA is a plain contiguous slice
(4-16 KB runs per partition).  The host also un-packs/upcasts the output.

Device algorithm (per core, T=4096, HD=H*D=512), bf16 data + fp32 PSUM:
  - T is split into NG=4 groups x CPG=8 chunks of 128 rows.
  - Main: per head, Y_local = (c1*L)^T-matmul(v) + (c2*L)^T-matmul(a) where
    L[p,q] = alpha^(p-q) (p>=q) is the within-chunk scan matrix.  Chunks are
    batched along the matmul free dim.
  - Level-2: chunk summaries s_c = Y_local[c][127] are scanned across chunks
    with small per-head matmuls using A = alpha^128 power matrices, yielding
    the carry P_c = S_{c-1} entering each chunk (S_{-1} = v0).
  - Fixup: rank-1 matmul decay_h (x) P_row accumulated into the output:
    y[128c+p] = Y_local[c][p] + alpha^(p+1) * P_c.
Cross-partition moves (chunk summaries -> partition-per-chunk, carries ->
single row) are small SBUF->SBUF DMAs; the group-to-group carry handoff is a
same-partition engine copy.
"""

import sys

sys.path.insert(0, "/opt/trn_rl_repo")

import ml_dtypes
import numpy as np

import concourse.bass as bass
import concourse.bacc as bacc
import concourse.mybir as mybir
from concourse.tile import TileContext
from concourse.bass_utils import run_bass_kernel_spmd

B, T, H, D = 8, 4096, 8, 64
HD = H * D                  # 512
P = 128                     # chunk length / partitions
NCHUNK = T // P             # 32
CPG = 8                     # chunks per group
NG = NCHUNK // CPG          # 4 groups
GT = CPG * P                # 1024 rows per group

F32 = mybir.dt.float32
BF16 = mybir.dt.bfloat16
NPBF = ml_dtypes.bfloat16


def build_consts(smoothing_weight, v0):
    """Host-side constant tensors (float64 math, cast to bf16)."""
    a = 1.0 / (1.0 + np.exp(-smoothing_weight.astype(np.float64).reshape(H)))
    c1 = 1.0 - a
    factor = c1 / np.maximum(c1, 1e-6)
    c2 = factor * a

    q = np.arange(P)
    e = q[None, :] - q[:, None]                     # [q, p] -> p - q
    pow_ = np.where(e >= 0, a[:, None, None] ** np.maximum(e, 0), 0.0)  # [h,q,p]
    w1 = (c1[:, None, None] * pow_).transpose(1, 0, 2).reshape(P, H * P)
    w2 = (c2[:, None, None] * pow_).transpose(1, 0, 2).reshape(P, H * P)

    decay = (a[:, None] ** (q[None, :] + 1)).reshape(1, H * P)  # [1, h*128]

    A = a ** P                                       # alpha^128 per head
    # Level-2 scan matrix with carry in ROW 0 (so the group-to-group carry
    # handoff is a partition-0 engine copy).  in[0]=carry_in, in[1+c]=s_c;
    # out[0]=carry_out=A^CPG*carry_in + sum_c A^(CPG-1-c) s_c,
    # out[1+j]=P_j=A^j*carry_in + sum_{c<j} A^(j-1-c) s_c.
    M = np.zeros((H, CPG + 1, CPG + 1))
    cidx = np.arange(CPG)
    for h in range(H):
        Ah = A[h]
        M[h, 0, 0] = Ah ** CPG
        M[h, 0, 1 + cidx] = Ah ** (CPG - 1 - cidx)
        for j in range(CPG):
            M[h, 1 + j, 0] = Ah ** j
            for c in range(j):
                M[h, 1 + j, 1 + c] = Ah ** (j - 1 - c)
    mexc = M.transpose(0, 2, 1)                      # [h, p, i] = lhsT per head
    mexc = mexc.transpose(1, 0, 2).reshape(CPG + 1, H * (CPG + 1))

    v0row = v0.astype(np.float64).reshape(1, HD)

    return {
        "w1": np.ascontiguousarray(w1).astype(NPBF),
        "w2": np.ascontiguousarray(w2).astype(NPBF),
        "decay": np.ascontiguousarray(decay).astype(NPBF),
        "mexc": np.ascontiguousarray(mexc).astype(NPBF),
        "v0r": np.ascontiguousarray(v0row).astype(NPBF),
    }


def pack_input(x):
    """[T, H, D] fp32 -> device layout [P, NCHUNK*HD] bf16 (partition = t%128)."""
    xt = x.reshape(NCHUNK, P, HD).transpose(1, 0, 2).reshape(P, NCHUNK * HD)
    return np.ascontiguousarray(xt).astype(NPBF)


def unpack_output(y):
    """Device layout [P, NCHUNK*HD] bf16 -> [T, H, D] fp32."""
    return np.ascontiguousarray(
        y.reshape(P, NCHUNK, HD).transpose(1, 0, 2)
    ).astype(np.float32).reshape(T, H, D)


def build_nc():
    nc = bacc.Bacc()

    v_d = nc.declare_dram_parameter("v", [P, NCHUNK * HD], BF16, isOutput=False)
    a_d = nc.declare_dram_parameter("a", [P, NCHUNK * HD], BF16, isOutput=False)
    w1_d = nc.declare_dram_parameter("w1", [P, H * P], BF16, isOutput=False)
    w2_d = nc.declare_dram_parameter("w2", [P, H * P], BF16, isOutput=False)
    dec_d = nc.declare_dram_parameter("decay", [1, H * P], BF16, isOutput=False)
    mex_d = nc.declare_dram_parameter("mexc", [CPG + 1, H * (CPG + 1)], BF16,
                                      isOutput=False)
    v0_d = nc.declare_dram_parameter("v0r", [1, HD], BF16, isOutput=False)
    y_d = nc.declare_dram_parameter("y", [P, NCHUNK * HD], BF16, isOutput=True)

    with TileContext(nc) as tc:
        with (
            tc.tile_pool(name="wpool", bufs=1) as wpool,
            tc.tile_pool(name="vin", bufs=3) as vin,
            tc.tile_pool(name="ain", bufs=3) as ain,
            tc.tile_pool(name="yout", bufs=3) as yout,
            tc.tile_pool(name="small", bufs=2) as small,
            tc.tile_pool(name="psA", bufs=4, space="PSUM") as psA_pool,
            tc.tile_pool(name="psP", bufs=1, space="PSUM") as psP_pool,
            tc.tile_pool(name="psB", bufs=3, space="PSUM") as psB_pool,
        ):
            # constants -> SBUF once
            w1 = wpool.tile([P, H * P], BF16, tag="w1")
            w2 = wpool.tile([P, H * P], BF16, tag="w2")
            dec = wpool.tile([1, H * P], BF16, tag="dec")
            mex = wpool.tile([CPG + 1, H * (CPG + 1)], BF16, tag="mex")
            nc.scalar.dma_start(w1[:], w1_d[:])
            nc.scalar.dma_start(w2[:], w2_d[:])
            nc.scalar.dma_start(dec[:], dec_d[:])
            nc.scalar.dma_start(mex[:], mex_d[:])

            p_sb_prev = None
            C4 = CPG // 2

            for g in range(NG):
                f0 = g * CPG * HD
                # ---- stream group inputs: contiguous [P, CPG*HD] slices,
                # half-group (4-chunk, 0.5 MB) granularity for overlap
                v_sb = vin.tile([P, CPG * HD], BF16, tag="v")
                a_sb = ain.tile([P, CPG * HD], BF16, tag="a")
                HF = C4 * HD
                for half in range(2):
                    fs = slice(half * HF, (half + 1) * HF)
                    src = slice(f0 + half * HF, f0 + (half + 1) * HF)
                    nc.sync.dma_start(v_sb[:, fs], v_d[:, src])
                    nc.sync.dma_start(a_sb[:, fs], a_d[:, src])

                y_sb = yout.tile([P, CPG * HD], BF16, tag="y")
                v3 = v_sb[:].rearrange("p (c m) -> p c m", c=CPG)
                a3 = a_sb[:].rearrange("p (c m) -> p c m", c=CPG)
                y3 = y_sb[:].rearrange("p (c m) -> p c m", c=CPG)

                # ---- main within-chunk scan, per head
                for h in range(H):
                    psA = psA_pool.tile([P, CPG * D], F32, tag="psA")
                    rhs_v = v3[:, :, h * D:(h + 1) * D]   # [P, CPG, D]
                    rhs_a = a3[:, :, h * D:(h + 1) * D]
                    if g == 0:
                        for half in range(2):
                            cs = slice(half * C4, (half + 1) * C4)
                            fs = slice(half * C4 * D, (half + 1) * C4 * D)
                            nc.tensor.matmul(psA[:, fs],
                                             w1[:, h * P:(h + 1) * P],
                                             rhs_v[:, cs, :],
                                             start=True, stop=False)
                            nc.tensor.matmul(psA[:, fs],
                                             w2[:, h * P:(h + 1) * P],
                                             rhs_a[:, cs, :],
                                             start=False, stop=True)
                    else:
                        nc.tensor.matmul(psA[:], w1[:, h * P:(h + 1) * P],
                                         rhs_v, start=True, stop=False)
                        nc.tensor.matmul(psA[:], w2[:, h * P:(h + 1) * P],
                                         rhs_a, start=False, stop=True)
                    # evacuate into interleaved (c, h, d) bf16 layout
                    nc.scalar.copy(y3[:, :, h * D:(h + 1) * D],
                                   psA[:].rearrange("p (c d) -> p c d", c=CPG))

                # ---- gather chunk summaries: row 127 -> [CPG, HD];
                #      row 8 = incoming carry state
                sT = small.tile([CPG + 1, HD], BF16, tag="sT")
                nc.gpsimd.dma_start(sT[1:CPG + 1, :], y_sb[P - 1:P, :])
                if g == 0:
                    nc.gpsimd.dma_start(sT[0:1, :], v0_d[:])
                else:
                    # same-partition (row 0 -> row 0): plain engine copy
                    nc.vector.tensor_copy(sT[0:1, :], p_sb_prev[0:1, :])

                # ---- level-2 scan: one augmented bf16 matmul per head
                psP = psP_pool.tile([CPG + 1, HD], F32, tag="psP")
                for h in range(H):
                    hs = slice(h * (CPG + 1), (h + 1) * (CPG + 1))
                    nc.tensor.matmul(psP[:, h * D:(h + 1) * D], mex[:, hs],
                                     sT[:, h * D:(h + 1) * D],
                                     start=True, stop=True)
                p_sb = small.tile([CPG + 1, HD], BF16, tag="p_sb")
                nc.scalar.copy(p_sb[:], psP[:])

                # ---- scatter carries to a single row [1, (c, h, d)]
                prow = small.tile([1, CPG * HD], BF16, tag="prow")
                nc.gpsimd.dma_start(prow[:], p_sb[1:CPG + 1, :])

                # ---- fixup: rank-1 decay (x) carry, then add into y
                prow3 = prow[:].rearrange("o (c m) -> o c m", c=CPG)
                for h in range(H):
                    psB = psB_pool.tile([P, CPG * D], F32, tag="psB")
                    nc.tensor.matmul(psB[:], dec[0:1, h * P:(h + 1) * P],
                                     prow3[:, :, h * D:(h + 1) * D],
                                     start=True, stop=True)
                    pb3 = psB[:].rearrange("p (c d) -> p c d", c=CPG)
                    for half in range(2):
                        cs = slice(half * C4, (half + 1) * C4)
                        yv = y3[:, cs, h * D:(h + 1) * D]
                        nc.vector.tensor_add(yv, yv, pb3[:, cs, :])
                # store per chunk-half: earlier, smoother out-stream
                for half in range(2):
                    fs = slice(half * HF, (half + 1) * HF)
                    dst = slice(f0 + half * HF, f0 + (half + 1) * HF)
                    nc.scalar.dma_start(y_d[:, dst], y_sb[:, fs])

                p_sb_prev = p_sb

    nc.finalize()
    return nc


_NC_CACHE = None


def _get_nc():
    global _NC_CACHE
    if _NC_CACHE is None:
        _NC_CACHE = build_nc()
    return _NC_CACHE


def kernel(values, aux_values, v0, smoothing_weight):
    consts = build_consts(smoothing_weight, v0)
    nc = _get_nc()
    in_maps = []
    for b in range(B):
        m = dict(consts)
        m["v"] = pack_input(np.asarray(values[b], dtype=np.float32))
        m["a"] = pack_input(np.asarray(aux_values[b], dtype=np.float32))
        in_maps.append(m)
    res = run_bass_kernel_spmd(nc, in_maps, list(range(B))).results
    out = np.stack([unpack_output(res[b]["y"]) for b in range(B)])
    return out.astype(np.float32)


# revision 3
# speedup vs baseline: 1.2467x; 1.0415x over previous
"""Exponential smoothing (linear recurrence scan) on 8 trn2 NeuronCores.

Math (per batch b, head h, dim d):
    alpha = sigmoid(smoothing_weight[h])
    u[t]  = (1-alpha)*values[t] + factor*alpha*aux_values[t]
    y[t]  = alpha*y[t-1] + u[t],   y[-1] = v0
Sharding: data-parallel over batch b -> 8 cores, one batch each.

v3: bf16 I/O + software-pipelined emission.
  - All HBM I/O is bf16 (12 MB/core); host packs inputs to the device-native
    layout [P=t%128, chunk, (h d)] so input DMAs are contiguous slices, and
    unpacks/upcasts the output.
  - The per-group serial chain (summary gather -> level-2 scan -> carry
    scatter -> fixup) is deferred across iterations: iteration g emits
    main-matmuls(g), level2(g-1), fixup/add/store(g-2).  The PE queue then
    never waits on the SBUF->SBUF carry DMAs, stays busy, and HAM keeps the
    clock at 2.4 GHz.
  - PSUM evacuations alternate DVE/ACT (ACT-only copies were 40% busy in v2).

Device algorithm (per core, T=4096, HD=H*D=512), bf16 data + fp32 PSUM:
  - T split into NG=4 groups x CPG=8 chunks of 128 rows.
  - Main: per head, Y_local = (c1*L)^T-matmul(v) + (c2*L)^T-matmul(a),
    L[p,q] = alpha^(p-q) (p>=q); chunks batched along the matmul free dim.
  - Level-2: chunk summaries s_c = Y_local[c][127] scanned across chunks via
    per-head [9,9] matmuls with A = alpha^128 powers (carry in row 0), giving
    the carry P_c entering each chunk (and the group carry-out in row 0).
  - Fixup: rank-1 matmul decay_h (x) P_row, added into the output:
    y[128c+p] = Y_local[c][p] + alpha^(p+1) * P_c.
"""

import sys

sys.path.insert(0, "/opt/trn_rl_repo")

import ml_dtypes
import numpy as np

import concourse.bass as bass
import concourse.bacc as bacc
import concourse.mybir as mybir
from concourse.tile import TileContext
from concourse.bass_utils import run_bass_kernel_spmd

B, T, H, D = 8, 4096, 8, 64
HD = H * D                  # 512
P = 128                     # chunk length / partitions
NCHUNK = T // P             # 32
CPG = 8                     # chunks per group
NG = NCHUNK // CPG          # 4 groups
GT = CPG * P                # 1024 rows per group

F32 = mybir.dt.float32
BF16 = mybir.dt.bfloat16
NPBF = ml_dtypes.bfloat16


def build_consts(smoothing_weight, v0):
    """Host-side constant tensors (float64 math, cast to bf16)."""
    a = 1.0 / (1.0 + np.exp(-smoothing_weight.astype(np.float64).reshape(H)))
    c1 = 1.0 - a
    factor = c1 / np.maximum(c1, 1e-6)
    c2 = factor * a

    q = np.arange(P)
    e = q[None, :] - q[:, None]                     # [q, p] -> p - q
    pow_ = np.where(e >= 0, a[:, None, None] ** np.maximum(e, 0), 0.0)  # [h,q,p]
    w1 = (c1[:, None, None] * pow_).transpose(1, 0, 2).reshape(P, H * P)
    w2 = (c2[:, None, None] * pow_).transpose(1, 0, 2).reshape(P, H * P)

    decay = (a[:, None] ** (q[None, :] + 1)).reshape(1, H * P)  # [1, h*128]

    A = a ** P                                       # alpha^128 per head
    # Level-2 scan matrix with carry in ROW 0 (so the group-to-group carry
    # handoff is a partition-0 engine copy).  in[0]=carry_in, in[1+c]=s_c;
    # out[0]=carry_out=A^CPG*carry_in + sum_c A^(CPG-1-c) s_c,
    # out[1+j]=P_j=A^j*carry_in + sum_{c<j} A^(j-1-c) s_c.
    M = np.zeros((H, CPG + 1, CPG + 1))
    cidx = np.arange(CPG)
    for h in range(H):
        Ah = A[h]
        M[h, 0, 0] = Ah ** CPG
        M[h, 0, 1 + cidx] = Ah ** (CPG - 1 - cidx)
        for j in range(CPG):
            M[h, 1 + j, 0] = Ah ** j
            for c in range(j):
                M[h, 1 + j, 1 + c] = Ah ** (j - 1 - c)
    mexc = M.transpose(0, 2, 1)                      # [h, p, i] = lhsT per head
    mexc = mexc.transpose(1, 0, 2).reshape(CPG + 1, H * (CPG + 1))

    v0row = v0.astype(np.float64).reshape(1, HD)

    return {
        "w1": np.ascontiguousarray(w1).astype(NPBF),
        "w2": np.ascontiguousarray(w2).astype(NPBF),
        "decay": np.ascontiguousarray(decay).astype(NPBF),
        "mexc": np.ascontiguousarray(mexc).astype(NPBF),
        "v0r": np.ascontiguousarray(v0row).astype(NPBF),
    }


def pack_input(x):
    """[T, H, D] fp32 -> device layout [P, NCHUNK*HD] bf16 (partition = t%128)."""
    xt = x.reshape(NCHUNK, P, HD).transpose(1, 0, 2).reshape(P, NCHUNK * HD)
    return np.ascontiguousarray(xt).astype(NPBF)


def unpack_output(y):
    """Device layout [P, NCHUNK*HD] bf16 -> [T, H, D] fp32."""
    return np.ascontiguousarray(
        y.reshape(P, NCHUNK, HD).transpose(1, 0, 2)
    ).astype(np.float32).reshape(T, H, D)


def build_nc():
    nc = bacc.Bacc()

    v_d = nc.declare_dram_parameter("v", [P, NCHUNK * HD], BF16, isOutput=False)
    a_d = nc.declare_dram_parameter("a", [P, NCHUNK * HD], BF16, isOutput=False)
    w1_d = nc.declare_dram_parameter("w1", [P, H * P], BF16, isOutput=False)
    w2_d = nc.declare_dram_parameter("w2", [P, H * P], BF16, isOutput=False)
    dec_d = nc.declare_dram_parameter("decay", [1, H * P], BF16, isOutput=False)
    mex_d = nc.declare_dram_parameter("mexc", [CPG + 1, H * (CPG + 1)], BF16,
                                      isOutput=False)
    v0_d = nc.declare_dram_parameter("v0r", [1, HD], BF16, isOutput=False)
    y_d = nc.declare_dram_parameter("y", [P, NCHUNK * HD], BF16, isOutput=True)

    with TileContext(nc) as tc:
        with (
            tc.tile_pool(name="wpool", bufs=1) as wpool,
            tc.tile_pool(name="vin", bufs=3) as vin,
            tc.tile_pool(name="ain", bufs=3) as ain,
            tc.tile_pool(name="yout", bufs=4) as yout,
            tc.tile_pool(name="small", bufs=3) as small,
            tc.tile_pool(name="psA", bufs=4, space="PSUM") as psA_pool,
            tc.tile_pool(name="psP", bufs=1, space="PSUM") as psP_pool,
            tc.tile_pool(name="psB", bufs=3, space="PSUM") as psB_pool,
        ):
            # constants -> SBUF once
            w1 = wpool.tile([P, H * P], BF16, tag="w1")
            w2 = wpool.tile([P, H * P], BF16, tag="w2")
            dec = wpool.tile([1, H * P], BF16, tag="dec")
            mex = wpool.tile([CPG + 1, H * (CPG + 1)], BF16, tag="mex")
            nc.scalar.dma_start(w1[:], w1_d[:])
            nc.scalar.dma_start(w2[:], w2_d[:])
            nc.scalar.dma_start(dec[:], dec_d[:])
            nc.scalar.dma_start(mex[:], mex_d[:])

            C4 = CPG // 2
            HF = C4 * HD

            # per-group state carried between pipeline stages
            st = [dict() for _ in range(NG)]

            def stage_load(g):
                f0 = g * CPG * HD
                v_sb = vin.tile([P, CPG * HD], BF16, tag="v")
                a_sb = ain.tile([P, CPG * HD], BF16, tag="a")
                for half in range(2):
                    fs = slice(half * HF, (half + 1) * HF)
                    src = slice(f0 + half * HF, f0 + (half + 1) * HF)
                    nc.sync.dma_start(v_sb[:, fs], v_d[:, src])
                    nc.sync.dma_start(a_sb[:, fs], a_d[:, src])
                st[g]["v_sb"], st[g]["a_sb"] = v_sb, a_sb

            def stage_main(g):
                v_sb, a_sb = st[g]["v_sb"], st[g]["a_sb"]
                y_sb = yout.tile([P, CPG * HD], BF16, tag="y")
                v3 = v_sb[:].rearrange("p (c m) -> p c m", c=CPG)
                a3 = a_sb[:].rearrange("p (c m) -> p c m", c=CPG)
                y3 = y_sb[:].rearrange("p (c m) -> p c m", c=CPG)
                for h in range(H):
                    psA = psA_pool.tile([P, CPG * D], F32, tag="psA")
                    rhs_v = v3[:, :, h * D:(h + 1) * D]   # [P, CPG, D]
                    rhs_a = a3[:, :, h * D:(h + 1) * D]
                    if g == 0:
                        for half in range(2):
                            cs = slice(half * C4, (half + 1) * C4)
                            fs = slice(half * C4 * D, (half + 1) * C4 * D)
                            nc.tensor.matmul(psA[:, fs],
                                             w1[:, h * P:(h + 1) * P],
                                             rhs_v[:, cs, :],
                                             start=True, stop=False)
                            nc.tensor.matmul(psA[:, fs],
                                             w2[:, h * P:(h + 1) * P],
                                             rhs_a[:, cs, :],
                                             start=False, stop=True)
                    else:
                        nc.tensor.matmul(psA[:], w1[:, h * P:(h + 1) * P],
                                         rhs_v, start=True, stop=False)
                        nc.tensor.matmul(psA[:], w2[:, h * P:(h + 1) * P],
                                         rhs_a, start=False, stop=True)
                    # evacuate into interleaved (c, h, d) bf16 layout;
                    # alternate DVE/ACT to split the copy load
                    dst = y3[:, :, h * D:(h + 1) * D]
                    src = psA[:].rearrange("p (c d) -> p c d", c=CPG)
                    if h % 2 == 0:
                        nc.vector.tensor_copy(dst, src)
                    else:
                        nc.scalar.copy(dst, src)
                st[g]["y_sb"] = y_sb

            def stage_gather(g):
                # row 127 of y -> sT rows 1..8 (chunk-per-partition)
                y_sb = st[g]["y_sb"]
                sT = small.tile([CPG + 1, HD], BF16, tag="sT")
                nc.gpsimd.dma_start(sT[1:CPG + 1, :], y_sb[P - 1:P, :])
                if g == 0:
                    nc.gpsimd.dma_start(sT[0:1, :], v0_d[:])
                st[g]["sT"] = sT

            def stage_carry_copy(g):
                # sT(g) row 0 <- carry-out of group g-1 (same partition 0)
                nc.vector.tensor_copy(st[g]["sT"][0:1, :],
                                      st[g - 1]["p_sb"][0:1, :])

            def stage_level2(g):
                sT = st[g]["sT"]
                psP = psP_pool.tile([CPG + 1, HD], F32, tag="psP")
                for h in range(H):
                    hs = slice(h * (CPG + 1), (h + 1) * (CPG + 1))
                    nc.tensor.matmul(psP[:, h * D:(h + 1) * D], mex[:, hs],
                                     sT[:, h * D:(h + 1) * D],
                                     start=True, stop=True)
                p_sb = small.tile([CPG + 1, HD], BF16, tag="p_sb")
                nc.scalar.copy(p_sb[:], psP[:])
                # scatter carries [c, (h d)] -> single row (c, h, d)
                prow = small.tile([1, CPG * HD], BF16, tag="prow")
                nc.gpsimd.dma_start(prow[:], p_sb[1:CPG + 1, :])
                st[g]["p_sb"], st[g]["prow"] = p_sb, prow

            def stage_fixup(g):
                f0 = g * CPG * HD
                y_sb = st[g]["y_sb"]
                prow = st[g]["prow"]
                y3 = y_sb[:].rearrange("p (c m) -> p c m", c=CPG)
                prow3 = prow[:].rearrange("o (c m) -> o c m", c=CPG)
                for h in range(H):
                    psB = psB_pool.tile([P, CPG * D], F32, tag="psB")
                    nc.tensor.matmul(psB[:], dec[0:1, h * P:(h + 1) * P],
                                     prow3[:, :, h * D:(h + 1) * D],
                                     start=True, stop=True)
                    yv = y3[:, :, h * D:(h + 1) * D]
                    nc.vector.tensor_add(yv, yv,
                                         psB[:].rearrange("p (c d) -> p c d",
                                                          c=CPG))
                nc.scalar.dma_start(y_d[:, f0:f0 + CPG * HD], y_sb[:])

            # ---- software pipeline: main(g) | level2(g-1) | fixup(g-2)
            for g in range(NG):
                stage_load(g)
                if g >= 2:
                    stage_carry_copy(g - 1)
                stage_main(g)
                stage_gather(g)
                if g >= 1:
                    stage_level2(g - 1)
                if g >= 2:
                    stage_fixup(g - 2)
            # epilogue
            stage_fixup(NG - 2)
            stage_carry_copy(NG - 1)
            stage_level2(NG - 1)
            stage_fixup(NG - 1)

    nc.finalize()
    return nc


_NC_CACHE = None


def _get_nc():
    global _NC_CACHE
    if _NC_CACHE is None:
        _NC_CACHE = build_nc()
    return _NC_CACHE


def kernel(values, aux_values, v0, smoothing_weight):
    consts = build_consts(smoothing_weight, v0)
    nc = _get_nc()
    in_maps = []
    for b in range(B):
        m = dict(consts)
        m["v"] = pack_input(np.asarray(values[b], dtype=np.float32))
        m["a"] = pack_input(np.asarray(aux_values[b], dtype=np.float32))
        in_maps.append(m)
    res = run_bass_kernel_spmd(nc, in_maps, list(range(B))).results
    out = np.stack([unpack_output(res[b]["y"]) for b in range(B)])
    return out.astype(np.float32)
